# revision 1
# baseline (speedup 1.0000x reference)
"""Two-layer GATv2 (BioGPT relation extractor GNN) on 8 Trainium2 NeuronCores.

Strategy (edge-parallel, dst-partitioned):
  - Pad nodes to 50176 = 8 * 6272; core k owns dst rows [k*6272, (k+1)*6272).
  - Edges (incl. self-loops) are sorted by dst and bucketed into 128-node
    "windows" (49 windows/core). Each window's edge list is padded to a
    multiple of 128 -> fixed B blocks per window (SPMD-uniform).
  - Node-parallel matmuls compute xl = x@Wl+bl, xr = x@Wr+br per core;
    xl is AllGather'd (sources are global), xr stays core-local.
  - Per edge block (128 edges): indirect-DMA gather xl[src] and xr[dst],
    msg = xl+xr, leaky_relu (Prelu alpha=0.2), alpha = reduce(msg*att).
    Per window: ex = exp(alpha) (global max bound 0 - range is tiny),
    one-hot selection matrices (is_equal vs iota) give segment sums via
    PE matmuls: denom = sel^T @ ex, out = (sel*ex)^T @ xl_gathered,
    accumulated in PSUM across the window's blocks.
  - out = relu(out * 1/(denom+1e-16) + bias); layer-1 output is stored
    transposed (PE transposes) so layer 2's matmul can consume it as lhsT.
"""

import numpy as np

import concourse.bass as bass
import concourse.mybir as mybir
from concourse import bacc
from concourse.tile import TileContext
from concourse.masks import make_identity
from concourse.bass_utils import run_bass_kernel_spmd

F32 = mybir.dt.float32
F16 = mybir.dt.float16
I32 = mybir.dt.int32
AT = mybir.ActivationFunctionType
OP = mybir.AluOpType

NCORES = 8
WIN = 128

# problem constants (hardcoded per harness contract)
N_NODES = 50000
D_IN = 1024
HID = 256
H1, H2 = 4, 1
NPC = 6272          # nodes per core (49 windows of 128)


def _cdiv(a, b):
    return -(-a // b)


# --------------------------------------------------------------------------
# device program
# --------------------------------------------------------------------------

def _mm_phase(tc, nc, pools, *, xT_dram, w_sb_view, bias_sb, npc, din, dout,
              out_left, out_right, dl):
    """y = x @ Wcat + bcat ; write y[:, :dl] -> out_left, y[:, dl:] -> out_right."""
    kc = din // 128
    mp, pp = pools
    for rt in range(npc // 128):
        xt = mp.tile([128, kc * 128], F16, tag="mm_xt")
        nc.sync.dma_start(
            out=xt[:].rearrange("p (a q) -> p a q", q=128),
            in_=xT_dram[:, rt * 128:(rt + 1) * 128].rearrange("(a p) q -> p a q", p=128),
        )
        ysb = mp.tile([128, dout], F32, tag="mm_y")
        nstep = min(512, dout)
        for nb_ in range(dout // nstep):
            ps = pp.tile([128, nstep], F32, tag="mm_ps")
            for k in range(kc):
                nc.tensor.matmul(
                    out=ps[:],
                    lhsT=xt[:].rearrange("p (a q) -> p a q", q=128)[:, k, :],
                    rhs=w_sb_view[:, k, nb_ * nstep:(nb_ + 1) * nstep],
                    start=(k == 0), stop=(k == kc - 1),
                )
            nc.scalar.copy(out=ysb[:, nb_ * nstep:(nb_ + 1) * nstep], in_=ps[:])
        ysb2 = mp.tile([128, dout], F16, tag="mm_y2")
        nc.vector.tensor_tensor(out=ysb2[:], in0=ysb[:], in1=bias_sb[:], op=OP.add)
        rows = slice(rt * 128, (rt + 1) * 128)
        nc.sync.dma_start(out=out_left[rows, :], in_=ysb2[:, 0:dl])
        nc.sync.dma_start(out=out_right[rows, :], in_=ysb2[:, dl:2 * dl])


def _edge_phase(tc, nc, pools, *, h, c, b1, nwpc, xl_full, xr_loc,
                att_sb, bias_sb, maskh_sb, src_sb, dstg_sb, dstl_sb,
                iota_sb, ident_sb, alpha_sb, out1T=None, out_ext=None,
                dbg_den=None, dbg_aw=None, dbg_exm=None, dbg_osb=None,
                dbg_sv0=None, dbg_xlt=None):
    d = h * c
    xlp, selp, xrp, tp, sp, svp, op_, pp, tpp = pools
    for w in range(nwpc):
        aw = sp.tile([128, h * b1], F32, tag="aw")
        xl_tiles = []
        sel_tiles = []
        for b in range(b1):
            col = w * b1 + b
            xlg = xlp.tile([128, d], F16, tag="xlg")
            nc.gpsimd.indirect_dma_start(
                out=xlg[:], out_offset=None, in_=xl_full[:],
                in_offset=bass.IndirectOffsetOnAxis(ap=src_sb[:, col:col + 1], axis=0))
            xrg = xrp.tile([128, d], F16, tag="xrg")
            nc.gpsimd.indirect_dma_start(
                out=xrg[:], out_offset=None, in_=xr_loc[:],
                in_offset=bass.IndirectOffsetOnAxis(ap=dstg_sb[:, col:col + 1], axis=0))
            msg = tp.tile([128, d], F16, tag="msg")
            nc.vector.tensor_tensor(out=msg[:], in0=xlg[:], in1=xrg[:], op=OP.add)
            lr = tp.tile([128, d], F16, tag="lr")
            nc.scalar.activation(out=lr[:], in_=msg[:], func=AT.Prelu,
                                 alpha=alpha_sb[:, :1])
            tm = tp.tile([128, d], F16, tag="tm")
            nc.vector.tensor_tensor(out=tm[:], in0=lr[:], in1=att_sb[:], op=OP.mult)
            if h > 1:
                nc.vector.tensor_reduce(
                    out=aw[:, b * h:(b + 1) * h],
                    in_=tm[:].rearrange("p (h c) -> p h c", c=c),
                    axis=mybir.AxisListType.X, op=OP.add)
            else:
                nc.vector.tensor_reduce(
                    out=aw[:, b:b + 1], in_=tm[:],
                    axis=mybir.AxisListType.X, op=OP.add)
            sel = selp.tile([128, 128], F16, tag="sel")
            nc.vector.tensor_tensor(
                out=sel[:], in0=dstl_sb[:, col:col + 1].to_broadcast([128, 128]),
                in1=iota_sb[:], op=OP.is_equal)
            xl_tiles.append(xlg)
            sel_tiles.append(sel)
        ex = sp.tile([128, h * b1], F32, tag="ex")
        nc.scalar.activation(out=ex[:], in_=aw[:], func=AT.Exp)
        exm = sp.tile([128, h * b1], F32, tag="exm")
        nc.vector.tensor_tensor(
            out=exm[:], in0=ex[:], in1=maskh_sb[:, w * h * b1:(w + 1) * h * b1],
            op=OP.mult)
        exm16 = sp.tile([128, h * b1], F16, tag="exm16")
        nc.vector.tensor_copy(out=exm16[:], in_=exm[:])
        den = pp.tile([128, h], F32, tag="den_ps")
        outp = pp.tile([128, d], F32, tag="out_ps")
        for b in range(b1):
            nc.tensor.matmul(out=den[:], lhsT=sel_tiles[b],
                             rhs=exm16[:, b * h:(b + 1) * h],
                             start=(b == 0), stop=(b == b1 - 1))
            if dbg_xlt is not None and w == 0 and b == 0:
                nc.sync.dma_start(out=dbg_xlt[:], in_=xl_tiles[b][:])
            for hh in range(h):
                sv = svp.tile([128, 128], F16, tag="sv")
                nc.vector.tensor_tensor(
                    out=sv[:], in0=sel_tiles[b],
                    in1=exm16[:, b * h + hh:b * h + hh + 1].to_broadcast([128, 128]),
                    op=OP.mult)
                if dbg_sv0 is not None and w == 0 and b == 0 and hh == 0:
                    nc.sync.dma_start(out=dbg_sv0[:], in_=sv[:])
                # start=True clears the ENTIRE psum bank, so only the head
                # group whose region begins a 2KB bank may issue it; later
                # head groups in the same bank accumulate onto the cleared
                # bank with start=False.
                st = (b == 0) and (hh * c * 4) % 2048 == 0
                nc.tensor.matmul(out=outp[:, hh * c:(hh + 1) * c], lhsT=sv[:],
                                 rhs=xl_tiles[b][:, hh * c:(hh + 1) * c],
                                 start=st, stop=(b == b1 - 1),
                                 skip_group_check=True)
        dene = sp.tile([128, h], F32, tag="dene")
        nc.vector.tensor_scalar_add(out=dene[:], in0=den[:], scalar1=1e-16)
        if dbg_den is not None:
            nc.sync.dma_start(out=dbg_den[w * 128:(w + 1) * 128, :], in_=dene[:])
        if dbg_aw is not None:
            nc.sync.dma_start(out=dbg_aw[w * 128:(w + 1) * 128, :], in_=aw[:])
        if dbg_exm is not None:
            nc.sync.dma_start(out=dbg_exm[w * 128:(w + 1) * 128, :], in_=exm[:])
        rden = sp.tile([128, h], F32, tag="rden")
        nc.vector.reciprocal(out=rden[:], in_=dene[:])
        osb = op_.tile([128, d], F32, tag="osb")
        for hh in range(h):
            nc.vector.tensor_tensor(
                out=osb[:, hh * c:(hh + 1) * c], in0=outp[:, hh * c:(hh + 1) * c],
                in1=rden[:, hh:hh + 1].to_broadcast([128, c]), op=OP.mult)
        if dbg_osb is not None:
            nc.sync.dma_start(out=dbg_osb[w * 128:(w + 1) * 128, :], in_=osb[:])
        osb2 = op_.tile([128, d], F32, tag="osb2")
        nc.vector.tensor_tensor(out=osb2[:], in0=osb[:], in1=bias_sb[:], op=OP.add)
        osb3 = op_.tile([128, d], F32, tag="osb3")
        nc.vector.tensor_scalar_max(out=osb3[:], in0=osb2[:], scalar1=0.0)
        if out1T is not None:
            for cc in range(d // 128):
                tps = tpp.tile([128, 128], F32, tag="tp_ps")
                nc.tensor.transpose(out=tps[:], in_=osb3[:, cc * 128:(cc + 1) * 128],
                                    identity=ident_sb[:])
                tsb = svp.tile([128, 128], F16, tag="tsb")
                nc.scalar.copy(out=tsb[:], in_=tps[:])
                nc.sync.dma_start(
                    out=out1T[cc * 128:(cc + 1) * 128, w * 128:(w + 1) * 128],
                    in_=tsb[:])
        else:
            nc.sync.dma_start(out=out_ext[w * 128:(w + 1) * 128, :], in_=osb3[:])


def build_program(*, npc, b1, din, hid, h1, h2, dbg=False, reps=1, rep_only=None):
    nwpc = npc // WIN
    nb = nwpc * b1
    d1, d2 = h1 * hid, h2 * hid
    npad = NCORES * npc

    nc = bacc.Bacc("TRN2", target_bir_lowering=False, debug=True)

    def param(name, shape, dt=F32):
        return nc.declare_dram_parameter(name, list(shape), dt, isOutput=False)

    xT = param("xT", [din, npc], F16)
    w1 = param("w1", [din, 2 * d1], F16)
    b1c = param("b1c", [128, 2 * d1])
    att1b = param("att1b", [128, d1], F16)
    bias1b = param("bias1b", [128, d1])
    w2 = param("w2", [d1, 2 * d2], F16)
    b2c = param("b2c", [128, 2 * d2])
    att2b = param("att2b", [128, d2], F16)
    bias2b = param("bias2b", [128, d2])
    iota = param("iota", [128, 128])
    srcT = param("srcT", [128, nb], I32)
    dstgT = param("dstgT", [128, nb], I32)
    dstlT = param("dstlT", [128, nb])
    mask1T = param("mask1T", [128, nb * h1])
    mask2T = param("mask2T", [128, nb * h2])
    out_ext = nc.declare_dram_parameter("out_ext", [npc, d2], F32, isOutput=True)
    if dbg:
        dbg_xl = nc.declare_dram_parameter("dbg_xl", [npc, d1], F16, isOutput=True)
        dbg_xr = nc.declare_dram_parameter("dbg_xr", [npc, d1], F16, isOutput=True)
        dbg_xlf = nc.declare_dram_parameter("dbg_xlf", [NCORES * npc, d1], F16, isOutput=True)
        dbg_o1t = nc.declare_dram_parameter("dbg_o1t", [d1, npc], F16, isOutput=True)
        dbg_den = nc.declare_dram_parameter("dbg_den", [npc, h1], F32, isOutput=True)
        dbg_aw = nc.declare_dram_parameter("dbg_aw", [npc, h1 * b1], F32, isOutput=True)
        dbg_exm = nc.declare_dram_parameter("dbg_exm", [npc, h1 * b1], F32, isOutput=True)
        dbg_osb = nc.declare_dram_parameter("dbg_osb", [npc, d1], F32, isOutput=True)
        dbg_sv0 = nc.declare_dram_parameter("dbg_sv0", [128, 128], F16, isOutput=True)
        dbg_xlt = nc.declare_dram_parameter("dbg_xlt", [128, d1], F16, isOutput=True)
        dbg_x2l = nc.declare_dram_parameter("dbg_x2l", [npc, d2], F16, isOutput=True)
        dbg_x2r = nc.declare_dram_parameter("dbg_x2r", [npc, d2], F16, isOutput=True)
        dbg_den2 = nc.declare_dram_parameter("dbg_den2", [npc, h2], F32, isOutput=True)
        dbg_aw2 = nc.declare_dram_parameter("dbg_aw2", [npc, h2 * b1], F32, isOutput=True)
        dbg_exm2 = nc.declare_dram_parameter("dbg_exm2", [npc, h2 * b1], F32, isOutput=True)

    xl_b = nc.dram_tensor("xl_b", [npc, d1], F16)
    xr_loc = nc.dram_tensor("xr_loc", [npc, d1], F16)
    xl_full = nc.dram_tensor("xl_full", [npad, d1], F16, addr_space="Shared")
    out1T = nc.dram_tensor("out1T", [d1, npc], F16)
    x2l_b = nc.dram_tensor("x2l_b", [npc, d2], F16)
    x2r_loc = nc.dram_tensor("x2r_loc", [npc, d2], F16)
    x2l_full = nc.dram_tensor("x2l_full", [npad, d2], F16, addr_space="Shared")

    rg = [list(range(NCORES))]

    with TileContext(nc) as tc:
        with tc.tile_pool(name="const", bufs=1) as cp:
            def load_const(ap, shape, dt=F32, tag=None):
                t = cp.tile(list(shape), dt, tag=tag)
                nc.sync.dma_start(out=t[:], in_=ap[:])
                return t

            src_sb = load_const(srcT, [128, nb], I32, tag="src_sb")
            dstg_sb = load_const(dstgT, [128, nb], I32, tag="dstg_sb")
            dstl_sb = load_const(dstlT, [128, nb], tag="dstl_sb")
            mask1_sb = load_const(mask1T, [128, nb * h1], tag="mask1_sb")
            mask2_sb = load_const(mask2T, [128, nb * h2], tag="mask2_sb")
            iota_sb = load_const(iota, [128, 128], tag="iota_sb")
            att1_sb = load_const(att1b, [128, d1], F16, tag="att1_sb")
            bias1_sb = load_const(bias1b, [128, d1], tag="bias1_sb")
            att2_sb = load_const(att2b, [128, d2], F16, tag="att2_sb")
            bias2_sb = load_const(bias2b, [128, d2], tag="bias2_sb")
            b1c_sb = load_const(b1c, [128, 2 * d1], tag="b1c_sb")
            b2c_sb = load_const(b2c, [128, 2 * d2], tag="b2c_sb")
            alpha_sb = cp.tile([128, 1], F32, tag="alpha_sb")
            nc.vector.memset(alpha_sb[:], 0.2)
            ident_sb = cp.tile([128, 128], F32, tag="ident_sb")
            make_identity(nc, ident_sb[:])

            def sec_mm1():
                # ---------------- layer 1 matmul ----------------
                kc1 = din // 128
                with (
                    tc.tile_pool(name="mmw", bufs=1) as wp,
                    tc.tile_pool(name="mm", bufs=3) as mp,
                    tc.tile_pool(name="mmp", bufs=4, space="PSUM") as pp,
                ):
                    w1_sb = wp.tile([128, kc1 * 2 * d1], F16, tag="w1_sb")
                    nc.sync.dma_start(
                        out=w1_sb[:].rearrange("p (a n) -> p a n", a=kc1),
                        in_=w1[:].rearrange("(a p) n -> p a n", p=128))
                    _mm_phase(tc, nc, (mp, pp), xT_dram=xT,
                              w_sb_view=w1_sb[:].rearrange("p (a n) -> p a n", a=kc1),
                              bias_sb=b1c_sb, npc=npc, din=din, dout=2 * d1,
                              out_left=xl_b, out_right=xr_loc, dl=d1)

            def sec_ag1():
                nc.gpsimd.collective_compute(
                    "AllGather", OP.bypass, replica_groups=rg,
                    ins=[xl_b[:]], outs=[xl_full[:]])

            def sec_edge1():
                # ---------------- layer 1 edge phase ----------------
                with (
                    tc.tile_pool(name="xlg", bufs=b1 + 2) as xlp,
                    tc.tile_pool(name="selp", bufs=b1 + 2) as selp,
                    tc.tile_pool(name="xrp", bufs=3) as xrp,
                    tc.tile_pool(name="tp", bufs=3) as tp,
                    tc.tile_pool(name="sp", bufs=3) as sp,
                    tc.tile_pool(name="svp", bufs=6) as svp,
                    tc.tile_pool(name="op", bufs=2) as op_,
                    tc.tile_pool(name="pp", bufs=2, space="PSUM") as ppp,
                    tc.tile_pool(name="tpp", bufs=2, space="PSUM") as tpp,
                ):
                    _edge_phase(tc, nc, (xlp, selp, xrp, tp, sp, svp, op_, ppp, tpp),
                                h=h1, c=hid, b1=b1, nwpc=nwpc,
                                xl_full=xl_full, xr_loc=xr_loc,
                                att_sb=att1_sb, bias_sb=bias1_sb, maskh_sb=mask1_sb,
                                src_sb=src_sb, dstg_sb=dstg_sb, dstl_sb=dstl_sb,
                                iota_sb=iota_sb, ident_sb=ident_sb, alpha_sb=alpha_sb,
                                out1T=out1T,
                                dbg_den=dbg_den if dbg else None,
                                dbg_aw=dbg_aw if dbg else None,
                                dbg_exm=dbg_exm if dbg else None,
                                dbg_osb=dbg_osb if dbg else None,
                                dbg_sv0=dbg_sv0 if dbg else None,
                                dbg_xlt=dbg_xlt if dbg else None)

            def sec_mm2():
                # ---------------- layer 2 matmul ----------------
                kc2 = d1 // 128
                with (
                    tc.tile_pool(name="mmw2", bufs=1) as wp2,
                    tc.tile_pool(name="mm2", bufs=3) as mp2,
                    tc.tile_pool(name="mmp2", bufs=4, space="PSUM") as pp2,
                ):
                    w2_sb = wp2.tile([128, kc2 * 2 * d2], F16, tag="w2_sb")
                    nc.sync.dma_start(
                        out=w2_sb[:].rearrange("p (a n) -> p a n", a=kc2),
                        in_=w2[:].rearrange("(a p) n -> p a n", p=128))
                    _mm_phase(tc, nc, (mp2, pp2), xT_dram=out1T,
                              w_sb_view=w2_sb[:].rearrange("p (a n) -> p a n", a=kc2),
                              bias_sb=b2c_sb, npc=npc, din=d1, dout=2 * d2,
                              out_left=x2l_b, out_right=x2r_loc, dl=d2)

            def sec_ag2():
                nc.gpsimd.collective_compute(
                    "AllGather", OP.bypass, replica_groups=rg,
                    ins=[x2l_b[:]], outs=[x2l_full[:]])

            def sec_edge2():
                # ---------------- layer 2 edge phase ----------------
                with (
                    tc.tile_pool(name="xlg2", bufs=b1 + 2) as xlp2,
                    tc.tile_pool(name="selp2", bufs=b1 + 2) as selp2,
                    tc.tile_pool(name="xrp2", bufs=3) as xrp2,
                    tc.tile_pool(name="tp2", bufs=3) as tp2,
                    tc.tile_pool(name="sp2", bufs=3) as sp2,
                    tc.tile_pool(name="svp2", bufs=6) as svp2,
                    tc.tile_pool(name="op2", bufs=2) as op2,
                    tc.tile_pool(name="pp2", bufs=2, space="PSUM") as ppp2,
                    tc.tile_pool(name="tpp2", bufs=1, space="PSUM") as tpp2,
                ):
                    _edge_phase(tc, nc, (xlp2, selp2, xrp2, tp2, sp2, svp2, op2, ppp2, tpp2),
                                h=h2, c=hid, b1=b1, nwpc=nwpc,
                                xl_full=x2l_full, xr_loc=x2r_loc,
                                att_sb=att2_sb, bias_sb=bias2_sb, maskh_sb=mask2_sb,
                                src_sb=src_sb, dstg_sb=dstg_sb, dstl_sb=dstl_sb,
                                iota_sb=iota_sb, ident_sb=ident_sb, alpha_sb=alpha_sb,
                                out_ext=out_ext,
                                dbg_den=dbg_den2 if dbg else None,
                                dbg_aw=dbg_aw2 if dbg else None,
                                dbg_exm=dbg_exm2 if dbg else None)

            sections = {"mm1": sec_mm1, "ag1": sec_ag1, "edge1": sec_edge1,
                        "mm2": sec_mm2, "ag2": sec_ag2, "edge2": sec_edge2}
            if rep_only is None:
                for _rep in range(reps):
                    for f in sections.values():
                        f()
            else:
                for f in sections.values():
                    f()
                for _rep in range(reps - 1):
                    sections[rep_only]()
            if dbg:
                nc.sync.dma_start(out=dbg_xl[:], in_=xl_b[:])
                nc.sync.dma_start(out=dbg_xr[:], in_=xr_loc[:])
                nc.sync.dma_start(out=dbg_xlf[:], in_=xl_full[:])
                nc.sync.dma_start(out=dbg_o1t[:], in_=out1T[:])
                nc.sync.dma_start(out=dbg_x2l[:], in_=x2l_b[:])
                nc.sync.dma_start(out=dbg_x2r[:], in_=x2r_loc[:])

    nc.finalize()
    return nc


# --------------------------------------------------------------------------
# host side
# --------------------------------------------------------------------------

def prepare(inputs, *, n, npc, din, hid, h1, h2):
    nwpc = npc // WIN
    npad = NCORES * npc
    d1, d2 = h1 * hid, h2 * hid

    x = np.asarray(inputs["node_features"], np.float32)
    ei = np.asarray(inputs["edge_index"])
    loops = np.arange(n, dtype=np.int64)
    src = np.concatenate([np.asarray(ei[0], np.int64), loops])
    dst = np.concatenate([np.asarray(ei[1], np.int64), loops])
    order = np.argsort(dst, kind="stable")
    srcs = src[order].astype(np.int32)
    dsts = dst[order].astype(np.int32)

    nwin_real = _cdiv(n, WIN)
    wg = dsts // WIN
    # re-sort within each window by src so each 128-edge gather block reads
    # a narrow src range (HBM locality for the indirect gathers)
    order2 = np.lexsort((srcs, wg))
    srcs = srcs[order2]
    dsts = dsts[order2]
    wg = wg[order2]
    cnt = np.bincount(wg, minlength=nwin_real)
    b1 = max(1, int(_cdiv(int(cnt.max()), WIN)))
    nb = nwpc * b1
    starts = np.zeros(nwin_real + 1, np.int64)
    starts[1:] = np.cumsum(cnt)
    j = np.arange(len(dsts), dtype=np.int64) - starts[wg]
    core = wg // nwpc
    wl = wg % nwpc
    col = wl * b1 + j // WIN
    row = j % WIN

    src_tab = np.zeros((NCORES, WIN, nb), np.int32)
    dstg_tab = np.zeros((NCORES, WIN, nb), np.int32)
    dstl_tab = np.zeros((NCORES, WIN, nb), np.float32)
    mask_tab = np.zeros((NCORES, WIN, nb), np.float32)
    src_tab[core, row, col] = srcs
    dstg_tab[core, row, col] = dsts - core.astype(np.int32) * npc
    dstl_tab[core, row, col] = (dsts % WIN).astype(np.float32)
    mask_tab[core, row, col] = 1.0
    mask1 = np.repeat(mask_tab, h1, axis=2)
    mask2 = np.repeat(mask_tab, h2, axis=2)

    x_pad = np.zeros((npad, din), np.float16)
    x_pad[:n] = x.astype(np.float16)
    xT_all = np.ascontiguousarray(x_pad.T)  # [din, npad]

    w1cat = np.concatenate([np.asarray(inputs["W1_l"], np.float32),
                            np.asarray(inputs["W1_r"], np.float32)], axis=1)
    b1cat = np.concatenate([np.asarray(inputs["b1_l"], np.float32),
                            np.asarray(inputs["b1_r"], np.float32)])
    w2cat = np.concatenate([np.asarray(inputs["W2_l"], np.float32),
                            np.asarray(inputs["W2_r"], np.float32)], axis=1)
    b2cat = np.concatenate([np.asarray(inputs["b2_l"], np.float32),
                            np.asarray(inputs["b2_r"], np.float32)])
    att1f = np.asarray(inputs["att1"], np.float32).reshape(-1)
    att2f = np.asarray(inputs["att2"], np.float32).reshape(-1)
    bias1 = np.asarray(inputs["bias1"], np.float32)
    bias2 = np.asarray(inputs["bias2"], np.float32)

    def bc(v):
        return np.ascontiguousarray(np.tile(np.asarray(v, np.float32)[None, :], (128, 1)))

    iota_row = np.ascontiguousarray(
        np.tile(np.arange(WIN, dtype=np.float32), (128, 1)))

    in_maps = []
    for k in range(NCORES):
        in_maps.append({
            "xT": np.ascontiguousarray(xT_all[:, k * npc:(k + 1) * npc]),
            "w1": w1cat.astype(np.float16), "b1c": bc(b1cat),
            "att1b": bc(att1f).astype(np.float16), "bias1b": bc(bias1),
            "w2": w2cat.astype(np.float16), "b2c": bc(b2cat),
            "att2b": bc(att2f).astype(np.float16), "bias2b": bc(bias2),
            "iota": iota_row,
            "srcT": np.ascontiguousarray(src_tab[k]),
            "dstgT": np.ascontiguousarray(dstg_tab[k]),
            "dstlT": np.ascontiguousarray(dstl_tab[k]),
            "mask1T": np.ascontiguousarray(mask1[k]),
            "mask2T": np.ascontiguousarray(mask2[k]),
        })
    return in_maps, b1


def gat_forward(inputs, *, n=N_NODES, npc=NPC, din=D_IN, hid=HID, h1=H1, h2=H2,
                runner=None, dbg=False, want_results=None):
    in_maps, b1 = prepare(inputs, n=n, npc=npc, din=din, hid=hid, h1=h1, h2=h2)
    nc = build_program(npc=npc, b1=b1, din=din, hid=hid, h1=h1, h2=h2, dbg=dbg)
    if runner is not None:
        results = runner(nc, in_maps)
    else:
        results = run_bass_kernel_spmd(nc, in_maps, list(range(NCORES))).results
    if want_results is not None:
        want_results.extend(results)
    out = np.concatenate([results[k]["out_ext"] for k in range(NCORES)], axis=0)
    return np.ascontiguousarray(out[:n])


def kernel(**inputs):
    return gat_forward(inputs)



# revision 5
# speedup vs baseline: 1.0387x; 1.0387x over previous
"""Two-layer GATv2 (BioGPT relation extractor GNN) on 8 Trainium2 NeuronCores.

Strategy (edge-parallel, dst-partitioned), v2:
  - Pad nodes to 50176 = 8 * 6272; core k owns dst rows [k*6272, (k+1)*6272).
  - Edges (incl. self-loops) are bucketed into 128-node dst windows (49 per
    core) and, within a window, split into a low-src stream (src < 32768) and
    a high-src stream so gather indices fit int16 (dma_gather requirement).
    Each stream is padded to whole 128-edge blocks (pad slots gather row 0 and
    carry dstl=255 so their one-hot sel column is all zero -> no mask needed).
  - Node-parallel matmuls (x @ [Wl|Wr] + b) run on pre-swizzled contiguous
    tiles; the bias is folded in as a K=1 matmul of a ones-row against the
    bias row. xl is AllGather'd (sources are global), xr stays core-local.
  - Per window: one dma_gather per stream for xl[src] (batched, ~1280 rows in
    one SWDGE instruction) plus one for xr[dst]; window-wide DVE/ACT ops
    compute leaky_relu(xl+xr)*att -> alpha -> exp (f16, with a -2 bias for
    range safety); one-hot sel matrices (is_equal vs iota) give segment sums
    on the PE: den = sel^T @ ex, out = (sel*ex_h)^T @ xl, PSUM-accumulated.
  - Layer-1 output is stored raw (pre-bias/relu) [node, 1024] f16; layer-2's
    matmul loads it with DMA-transpose and applies relu(x+bias) on the scalar
    engine per 128-channel chunk (bias is per-partition there).
"""

import numpy as np

import concourse.bass as bass
import concourse.mybir as mybir
from concourse import bacc
from concourse.tile import TileContext
from concourse.bass_utils import run_bass_kernel_spmd

F32 = mybir.dt.float32
F16 = mybir.dt.float16
I16 = mybir.dt.int16
AT = mybir.ActivationFunctionType
OP = mybir.AluOpType

NCORES = 8
WIN = 128

N_NODES = 50000
D_IN = 1024
HID = 256
H1, H2 = 4, 1
NPC = 6272          # nodes per core (49 windows of 128)
NW = NPC // WIN     # 49
NPAD = NCORES * NPC
SPLIT = 32768       # int16 gather-index split point
D1 = H1 * HID
D2 = H2 * HID


def _cdiv(a, b):
    return -(-a // b)


# --------------------------------------------------------------------------
# device program
# --------------------------------------------------------------------------

def _mm_phase(tc, nc, pools, *, x_dram, w_sb, bias_row_sb, ones_sb, dout, dl,
              out_left, out_right, transpose_in=False, act_bias_sb=None):
    """y = act(x) @ Wcat + bcat ; y[:, :dl] -> out_left, y[:, dl:] -> out_right.

    transpose_in: x_dram rows are [node, ch]; load with DMA-transpose so the
    SBUF tile is [ch_in_chunk(p), chunk(a), node(q)], then apply
    relu(x + bias) per chunk on ACT (act_bias_sb is [128, 8], col a holding
    bias[a*128+p])."""
    mp, pp = pools
    kc = 8
    for rt in range(NW):
        rows = slice(rt * 128, (rt + 1) * 128)
        if transpose_in:
            xraw = mp.tile([128, kc * 128], F16, tag="mm_xraw")
            nc.sync.dma_start(
                out=xraw[:].rearrange("p (a q) -> p a q", q=128),
                in_=x_dram[rows, :], transpose=True)
            xt = mp.tile([128, kc * 128], F16, tag="mm_xt")
            for a in range(kc):
                nc.scalar.activation(
                    out=xt[:, a * 128:(a + 1) * 128],
                    in_=xraw[:, a * 128:(a + 1) * 128],
                    func=AT.Relu, bias=act_bias_sb[:, a:a + 1])
        else:
            xt = mp.tile([128, kc * 128], F16, tag="mm_xt")
            nc.sync.dma_start(out=xt[:], in_=x_dram[rows, :])
        ysb = mp.tile([128, dout], F16, tag="mm_y")
        nstep = min(512, dout)
        for nb_ in range(dout // nstep):
            cols = slice(nb_ * nstep, (nb_ + 1) * nstep)
            ps = pp.tile([128, nstep], F32, tag="mm_ps")
            nc.tensor.matmul(out=ps[:], lhsT=ones_sb[:],
                             rhs=bias_row_sb[:, cols], start=True, stop=False)
            for k in range(kc):
                nc.tensor.matmul(
                    out=ps[:], lhsT=xt[:, k * 128:(k + 1) * 128],
                    rhs=w_sb[:].rearrange("p (a n) -> p a n", a=kc)[:, k, cols],
                    start=False, stop=(k == kc - 1))
            nc.scalar.copy(out=ysb[:, cols], in_=ps[:])
        nc.scalar.dma_start(out=out_left[rows, :], in_=ysb[:, 0:dl])
        nc.scalar.dma_start(out=out_right[rows, :], in_=ysb[:, dl:2 * dl])


def _edge_phase(tc, nc, pools, meta, *, h, c, xl_full, xr_loc, attB_sb,
                bias_sb, dstl_sb, iota_sb, alpha_sb, nbias_sb,
                lo_sb, hi_sb, xr_sb, out_raw=None, out_ext=None, dbg=None):
    d = h * c
    xlp, xrp, selp, svp, sp, op_, pp = pools
    NLO, NHI, B, C, CLO, CHI, CB, Bmax = (
        meta["NLO"], meta["NHI"], meta["B"], meta["C"],
        meta["CLO"], meta["CHI"], meta["CB"], meta["Bmax"])
    for w in range(NW):
        nlo, nhi, b_, coff = NLO[w], NHI[w], B[w], C[w]
        xlg = xlp.tile([128, Bmax * d], F16, tag="xlg")
        xlg3 = xlg[:].rearrange("p (b e) -> p b e", e=d)
        if nhi:
            nc.gpsimd.dma_gather(
                out_ap=xlg3[:, nlo:b_, :], in_ap=xl_full[SPLIT:, :],
                idxs_ap=hi_sb[:, CHI[w]:CHI[w] + nhi * 8],
                num_idxs=nhi * 128, num_idxs_reg=nhi * 128, elem_size=d,
                single_packet=False)
        if nlo:
            nc.gpsimd.dma_gather(
                out_ap=xlg3[:, 0:nlo, :], in_ap=xl_full[:],
                idxs_ap=lo_sb[:, CLO[w]:CLO[w] + nlo * 8],
                num_idxs=nlo * 128, num_idxs_reg=nlo * 128, elem_size=d,
                single_packet=False)
        xrg = xrp.tile([128, Bmax * d], F16, tag="xrg")
        xrg3 = xrg[:].rearrange("p (b e) -> p b e", e=d)
        nc.gpsimd.dma_gather(
            out_ap=xrg3[:, 0:b_, :], in_ap=xr_loc[:],
            idxs_ap=xr_sb[:, CB[w] * 8:(CB[w] + b_) * 8],
            num_idxs=b_ * 128, num_idxs_reg=b_ * 128, elem_size=d,
            single_packet=False)
        # msg = xl + xr (into xrg), leaky, *att  -- all window-wide, in place
        nc.vector.tensor_tensor(out=xrg[:, :b_ * d], in0=xlg[:, :b_ * d],
                                in1=xrg[:, :b_ * d], op=OP.add)
        nc.scalar.activation(out=xrg[:, :b_ * d], in_=xrg[:, :b_ * d],
                             func=AT.Prelu, alpha=alpha_sb[:, :1])
        nc.vector.tensor_tensor(out=xrg[:, :b_ * d], in0=xrg[:, :b_ * d],
                                in1=attB_sb[:, :b_ * d], op=OP.mult)
        aw = sp.tile([128, Bmax * h], F32, tag="aw")
        nc.vector.tensor_reduce(
            out=aw[:, :b_ * h],
            in_=xrg[:, :b_ * d].rearrange("p (g c) -> p g c", c=c),
            axis=mybir.AxisListType.X, op=OP.add)
        exm = sp.tile([128, Bmax * h], F16, tag="exm")
        nc.scalar.activation(out=exm[:, :b_ * h], in_=aw[:, :b_ * h],
                             func=AT.Exp, bias=nbias_sb[:, :1])
        sel = selp.tile([128, Bmax * WIN], F16, tag="sel")
        nc.vector.tensor_tensor(
            out=sel[:, :b_ * WIN].rearrange("p (b q) -> p b q", q=WIN),
            in0=dstl_sb[:, coff:coff + b_]
                .rearrange("p (b o) -> p b o", o=1).to_broadcast([128, b_, WIN]),
            in1=iota_sb[:].rearrange("p (o q) -> p o q", o=1)
                .to_broadcast([128, b_, WIN]),
            op=OP.is_equal)
        den = pp.tile([128, h], F32, tag="den_ps")
        outp = pp.tile([128, d], F32, tag="out_ps")
        for b in range(b_):
            selb = sel[:, b * WIN:(b + 1) * WIN]
            nc.tensor.matmul(out=den[:], lhsT=selb,
                             rhs=exm[:, b * h:(b + 1) * h],
                             start=(b == 0), stop=(b == b_ - 1))
            sv = svp.tile([128, h * WIN], F16, tag="sv")
            if h > 1:
                nc.vector.tensor_tensor(
                    out=sv[:].rearrange("p (g q) -> p g q", q=WIN),
                    in0=selb.rearrange("p (o q) -> p o q", o=1)
                        .to_broadcast([128, h, WIN]),
                    in1=exm[:, b * h:(b + 1) * h]
                        .rearrange("p (g o) -> p g o", o=1)
                        .to_broadcast([128, h, WIN]),
                    op=OP.mult)
            else:
                nc.vector.tensor_tensor(
                    out=sv[:], in0=selb,
                    in1=exm[:, b:b + 1].to_broadcast([128, WIN]), op=OP.mult)
            for hh in range(h):
                st = (b == 0) and (hh * c * 4) % 2048 == 0
                nc.tensor.matmul(
                    out=outp[:, hh * c:(hh + 1) * c],
                    lhsT=sv[:, hh * WIN:(hh + 1) * WIN],
                    rhs=xlg[:, b * d + hh * c:b * d + (hh + 1) * c],
                    start=st, stop=(b == b_ - 1), skip_group_check=True)
        dene = sp.tile([128, h], F32, tag="dene")
        nc.vector.tensor_scalar_add(out=dene[:], in0=den[:], scalar1=1e-16)
        rden = sp.tile([128, h], F32, tag="rden")
        nc.vector.reciprocal(out=rden[:], in_=dene[:])
        rows = slice(w * 128, (w + 1) * 128)
        if dbg is not None and "den" in dbg:
            nc.sync.dma_start(out=dbg["den"][rows, :], in_=dene[:])
        if dbg is not None and "aw" in dbg:
            nc.sync.dma_start(out=dbg["aw"][rows, :b_ * h], in_=aw[:, :b_ * h])
        if out_raw is not None:
            osb = op_.tile([128, d], F16, tag="osb")
            for hh in range(h):
                nc.vector.tensor_tensor(
                    out=osb[:, hh * c:(hh + 1) * c],
                    in0=outp[:, hh * c:(hh + 1) * c],
                    in1=rden[:, hh:hh + 1].to_broadcast([128, c]), op=OP.mult)
            nc.scalar.dma_start(out=out_raw[rows, :], in_=osb[:])
        else:
            osb = op_.tile([128, d], F32, tag="osb")
            for hh in range(h):
                nc.vector.tensor_tensor(
                    out=osb[:, hh * c:(hh + 1) * c],
                    in0=outp[:, hh * c:(hh + 1) * c],
                    in1=rden[:, hh:hh + 1].to_broadcast([128, c]), op=OP.mult)
            nc.vector.tensor_tensor(out=osb[:], in0=osb[:], in1=bias_sb[:],
                                    op=OP.add)
            osb2 = op_.tile([128, d], F32, tag="osb2")
            nc.vector.tensor_scalar_max(out=osb2[:], in0=osb[:], scalar1=0.0)
            nc.scalar.dma_start(out=out_ext[rows, :], in_=osb2[:])


def build_program(meta, *, dbg=False, reps=1, rep_only=None, only=None):
    Bmax = meta["Bmax"]
    nblk = meta["NBLK"]
    clo_t = meta["CLO_T"]
    chi_t = meta["CHI_T"]

    nc = bacc.Bacc("TRN2", target_bir_lowering=False, debug=True)

    def param(name, shape, dt=F32):
        return nc.declare_dram_parameter(name, list(shape), dt, isOutput=False)

    xt_d = param("xt", [NPC, D_IN], F16)
    w1t = param("w1t", [128, 8 * 2 * D1], F16)
    b1r = param("b1r", [1, 2 * D1], F16)
    w2t = param("w2t", [128, 8 * 2 * D2], F16)
    b2r = param("b2r", [1, 2 * D2], F16)
    att1B = param("att1B", [128, Bmax * D1], F16)
    att2B = param("att2B", [128, Bmax * D2], F16)
    bias1c = param("bias1c", [128, 8])
    bias2b = param("bias2b", [128, D2])
    iota = param("iota", [128, 128])
    dstlT = param("dstlT", [128, nblk])
    xlloT = param("xlloT", [128, clo_t], I16)
    xlhiT = param("xlhiT", [128, chi_t], I16)
    xrT = param("xrT", [128, nblk * 8], I16)
    out_ext = nc.declare_dram_parameter("out_ext", [NPC, D2], F32, isOutput=True)
    dbg_t = {}
    if dbg:
        dbg_t["xl"] = nc.declare_dram_parameter("dbg_xl", [NPC, D1], F16, isOutput=True)
        dbg_t["xr"] = nc.declare_dram_parameter("dbg_xr", [NPC, D1], F16, isOutput=True)
        dbg_t["o1"] = nc.declare_dram_parameter("dbg_o1", [NPC, D1], F16, isOutput=True)
        dbg_t["den"] = nc.declare_dram_parameter("dbg_den", [NPC, H1], F32, isOutput=True)
        dbg_t["aw"] = nc.declare_dram_parameter("dbg_aw", [NPC, Bmax * H1], F32, isOutput=True)
        dbg_t["x2l"] = nc.declare_dram_parameter("dbg_x2l", [NPC, D2], F16, isOutput=True)
        dbg_t["den2"] = nc.declare_dram_parameter("dbg_den2", [NPC, H2], F32, isOutput=True)

    xl_b = nc.dram_tensor("xl_b", [NPC, D1], F16)
    xr_loc = nc.dram_tensor("xr_loc", [NPC, D1], F16)
    xl_full = nc.dram_tensor("xl_full", [NPAD, D1], F16, addr_space="Shared")
    out1 = nc.dram_tensor("out1", [NPC, D1], F16)
    x2l_b = nc.dram_tensor("x2l_b", [NPC, D2], F16)
    x2r_loc = nc.dram_tensor("x2r_loc", [NPC, D2], F16)
    x2l_full = nc.dram_tensor("x2l_full", [NPAD, D2], F16, addr_space="Shared")

    rg = [list(range(NCORES))]

    with TileContext(nc) as tc:
        with tc.tile_pool(name="const", bufs=1) as cp:
            def load_const(ap, shape, dt=F32, tag=None, eng=None):
                t = cp.tile(list(shape), dt, tag=tag)
                (eng or nc.sync).dma_start(out=t[:], in_=ap[:])
                return t

            lo_sb = load_const(xlloT, [128, clo_t], I16, tag="lo_sb")
            hi_sb = load_const(xlhiT, [128, chi_t], I16, tag="hi_sb")
            xr_sb = load_const(xrT, [128, nblk * 8], I16, tag="xr_sb")
            dstl_sb = load_const(dstlT, [128, nblk], tag="dstl_sb")
            iota_sb = load_const(iota, [128, 128], tag="iota_sb")
            att1B_sb = load_const(att1B, [128, Bmax * D1], F16, tag="att1B_sb")
            att2B_sb = load_const(att2B, [128, Bmax * D2], F16, tag="att2B_sb")
            bias1c_sb = load_const(bias1c, [128, 8], tag="bias1c_sb")
            bias2b_sb = load_const(bias2b, [128, D2], tag="bias2b_sb")
            b1r_sb = load_const(b1r, [1, 2 * D1], F16, tag="b1r_sb")
            b2r_sb = load_const(b2r, [1, 2 * D2], F16, tag="b2r_sb")
            alpha_sb = cp.tile([128, 1], F32, tag="alpha_sb")
            nc.vector.memset(alpha_sb[:], 0.2)
            nbias_sb = cp.tile([128, 1], F32, tag="nbias_sb")
            nc.vector.memset(nbias_sb[:], -2.0)
            ones_sb = cp.tile([1, 128], F16, tag="ones_sb")
            nc.vector.memset(ones_sb[:], 1.0)

            def sec_mm1():
                with (
                    tc.tile_pool(name="mmw", bufs=1) as wp,
                    tc.tile_pool(name="mm", bufs=3) as mp,
                    tc.tile_pool(name="mmp", bufs=4, space="PSUM") as pp,
                ):
                    w1_sb = wp.tile([128, 8 * 2 * D1], F16, tag="w1_sb")
                    nc.sync.dma_start(out=w1_sb[:], in_=w1t[:])
                    _mm_phase(tc, nc, (mp, pp), x_dram=xt_d, w_sb=w1_sb,
                              bias_row_sb=b1r_sb, ones_sb=ones_sb,
                              dout=2 * D1, dl=D1,
                              out_left=xl_b, out_right=xr_loc)

            def sec_ag1():
                nc.gpsimd.collective_compute(
                    "AllGather", OP.bypass, replica_groups=rg,
                    ins=[xl_b[:]], outs=[xl_full[:]])

            def sec_edge1():
                with (
                    tc.tile_pool(name="xlg1", bufs=2) as xlp,
                    tc.tile_pool(name="xrg1", bufs=2) as xrp,
                    tc.tile_pool(name="sel1", bufs=2) as selp,
                    tc.tile_pool(name="sv1", bufs=3) as svp,
                    tc.tile_pool(name="sp1", bufs=3) as sp,
                    tc.tile_pool(name="op1", bufs=2) as op_,
                    tc.tile_pool(name="pp1", bufs=2, space="PSUM") as pp,
                ):
                    _edge_phase(tc, nc, (xlp, xrp, selp, svp, sp, op_, pp),
                                meta, h=H1, c=HID,
                                xl_full=xl_full, xr_loc=xr_loc,
                                attB_sb=att1B_sb, bias_sb=None,
                                dstl_sb=dstl_sb, iota_sb=iota_sb,
                                alpha_sb=alpha_sb, nbias_sb=nbias_sb,
                                lo_sb=lo_sb, hi_sb=hi_sb, xr_sb=xr_sb,
                                out_raw=out1,
                                dbg=dbg_t if dbg else None)

            def sec_mm2():
                with (
                    tc.tile_pool(name="mmw2", bufs=1) as wp2,
                    tc.tile_pool(name="mm2", bufs=3) as mp2,
                    tc.tile_pool(name="mmp2", bufs=4, space="PSUM") as pp2,
                ):
                    w2_sb = wp2.tile([128, 8 * 2 * D2], F16, tag="w2_sb")
                    nc.sync.dma_start(out=w2_sb[:], in_=w2t[:])
                    _mm_phase(tc, nc, (mp2, pp2), x_dram=out1, w_sb=w2_sb,
                              bias_row_sb=b2r_sb, ones_sb=ones_sb,
                              dout=2 * D2, dl=D2,
                              out_left=x2l_b, out_right=x2r_loc,
                              transpose_in=True, act_bias_sb=bias1c_sb)

            def sec_ag2():
                nc.gpsimd.collective_compute(
                    "AllGather", OP.bypass, replica_groups=rg,
                    ins=[x2l_b[:]], outs=[x2l_full[:]])

            def sec_edge2():
                with (
                    tc.tile_pool(name="xlg2", bufs=2) as xlp2,
                    tc.tile_pool(name="xrg2", bufs=2) as xrp2,
                    tc.tile_pool(name="sel2", bufs=2) as selp2,
                    tc.tile_pool(name="sv2", bufs=3) as svp2,
                    tc.tile_pool(name="sp2", bufs=3) as sp2,
                    tc.tile_pool(name="op2", bufs=2) as op2,
                    tc.tile_pool(name="pp2", bufs=2, space="PSUM") as pp2,
                ):
                    dbg2 = None
                    if dbg:
                        dbg2 = {"den": dbg_t["den2"]}
                    _edge_phase(tc, nc, (xlp2, xrp2, selp2, svp2, sp2, op2, pp2),
                                meta, h=H2, c=HID,
                                xl_full=x2l_full, xr_loc=x2r_loc,
                                attB_sb=att2B_sb, bias_sb=bias2b_sb,
                                dstl_sb=dstl_sb, iota_sb=iota_sb,
                                alpha_sb=alpha_sb, nbias_sb=nbias_sb,
                                lo_sb=lo_sb, hi_sb=hi_sb, xr_sb=xr_sb,
                                out_ext=out_ext, dbg=dbg2)

            sections = {"mm1": sec_mm1, "ag1": sec_ag1, "edge1": sec_edge1,
                        "mm2": sec_mm2, "ag2": sec_ag2, "edge2": sec_edge2}
            if only is not None:
                sections = {k: v for k, v in sections.items() if k in only}
            if rep_only is None:
                for _rep in range(reps):
                    for f in sections.values():
                        f()
            else:
                for f in sections.values():
                    f()
                for _rep in range(reps - 1):
                    sections[rep_only]()
            if dbg:
                nc.sync.dma_start(out=dbg_t["xl"][:], in_=xl_b[:])
                nc.sync.dma_start(out=dbg_t["xr"][:], in_=xr_loc[:])
                nc.sync.dma_start(out=dbg_t["o1"][:], in_=out1[:])
                nc.sync.dma_start(out=dbg_t["x2l"][:], in_=x2l_b[:])

    nc.finalize()
    return nc


# --------------------------------------------------------------------------
# host side
# --------------------------------------------------------------------------

def _wrap16(idx_list, ncols):
    """Wrap an index list into the [16, ncols] layout, replicated to 128 rows."""
    out = np.zeros((16, ncols), np.int16)
    n = len(idx_list)
    if n:
        flat = np.zeros(16 * ncols, np.int16)
        flat[:n] = idx_list
        out = flat.reshape(ncols, 16).T.copy()
    return np.tile(out, (8, 1))


def prepare(inputs):
    x = np.asarray(inputs["node_features"], np.float32)
    ei = np.asarray(inputs["edge_index"])
    n = N_NODES
    loops = np.arange(n, dtype=np.int64)
    src = np.concatenate([np.asarray(ei[0], np.int64), loops]).astype(np.int64)
    dst = np.concatenate([np.asarray(ei[1], np.int64), loops]).astype(np.int64)
    wg = dst // WIN
    order = np.lexsort((src, wg))
    srcs = src[order].astype(np.int32)
    dsts = dst[order].astype(np.int32)
    wgs = wg[order]

    nwin = NCORES * NW
    cnt = np.bincount(wgs, minlength=nwin)
    starts = np.zeros(nwin + 1, np.int64)
    starts[1:] = np.cumsum(cnt)
    # per (core, window): lo/hi counts
    lo_cnt = np.zeros((NCORES, NW), np.int64)
    for g in range(nwin):
        k, w = g // NW, g % NW
        s, e = starts[g], starts[g + 1]
        lo_cnt[k, w] = np.searchsorted(srcs[s:e], SPLIT)
    hi_cnt = cnt.reshape(NCORES, NW) - lo_cnt

    NLO = _cdiv(lo_cnt.max(axis=0), 128).astype(int)
    NHI = _cdiv(hi_cnt.max(axis=0), 128).astype(int)
    B = NLO + NHI
    C = np.zeros(NW, int)
    C[1:] = np.cumsum(B)[:-1]
    CLO = np.zeros(NW, int)
    CLO[1:] = np.cumsum(NLO * 8)[:-1]
    CHI = np.zeros(NW, int)
    CHI[1:] = np.cumsum(NHI * 8)[:-1]
    NBLK = int(B.sum())
    Bmax = int(B.max())
    meta = {"NLO": NLO.tolist(), "NHI": NHI.tolist(), "B": B.tolist(),
            "C": C.tolist(), "CLO": CLO.tolist(), "CHI": CHI.tolist(),
            "CB": C.tolist(), "Bmax": Bmax, "NBLK": NBLK,
            "CLO_T": int((NLO * 8).sum()), "CHI_T": int((NHI * 8).sum())}

    x_pad = np.zeros((NPAD, D_IN), np.float16)
    x_pad[:n] = x.astype(np.float16)

    w1cat = np.concatenate([np.asarray(inputs["W1_l"], np.float32),
                            np.asarray(inputs["W1_r"], np.float32)], axis=1)
    b1cat = np.concatenate([np.asarray(inputs["b1_l"], np.float32),
                            np.asarray(inputs["b1_r"], np.float32)])
    w2cat = np.concatenate([np.asarray(inputs["W2_l"], np.float32),
                            np.asarray(inputs["W2_r"], np.float32)], axis=1)
    b2cat = np.concatenate([np.asarray(inputs["b2_l"], np.float32),
                            np.asarray(inputs["b2_r"], np.float32)])
    att1f = np.asarray(inputs["att1"], np.float32).reshape(-1)
    att2f = np.asarray(inputs["att2"], np.float32).reshape(-1)
    bias1 = np.asarray(inputs["bias1"], np.float32)
    bias2 = np.asarray(inputs["bias2"], np.float32)

    w1t = np.ascontiguousarray(
        w1cat.astype(np.float16).reshape(8, 128, 2 * D1)
        .transpose(1, 0, 2).reshape(128, 8 * 2 * D1))
    w2t_src = w2cat.astype(np.float16)  # [D1, 2*D2]
    w2t = np.ascontiguousarray(
        w2t_src.reshape(8, 128, 2 * D2).transpose(1, 0, 2).reshape(128, 8 * 2 * D2))

    att1Brow = np.tile(att1f.astype(np.float16), Bmax)
    att2Brow = np.tile(att2f.astype(np.float16), Bmax)
    att1B = np.ascontiguousarray(np.tile(att1Brow[None, :], (128, 1)))
    att2B = np.ascontiguousarray(np.tile(att2Brow[None, :], (128, 1)))
    bias1c = np.ascontiguousarray(
        bias1.reshape(8, 128).T.astype(np.float32))
    bias2b = np.ascontiguousarray(np.tile(bias2[None, :], (128, 1)).astype(np.float32))
    iota_row = np.ascontiguousarray(
        np.tile(np.arange(WIN, dtype=np.float32), (128, 1)))

    in_maps = []
    for k in range(NCORES):
        xc = x_pad[k * NPC:(k + 1) * NPC]  # [NPC, D_IN] f16
        xt = np.ascontiguousarray(
            xc.reshape(NW, 128, 8, 128).transpose(0, 3, 2, 1)
            .reshape(NPC, D_IN))

        dstl = np.full((128, NBLK), 255.0, np.float32)
        lo_tab = np.zeros((128, meta["CLO_T"]), np.int16)
        hi_tab = np.zeros((128, meta["CHI_T"]), np.int16)
        xr_tab = np.zeros((128, NBLK * 8), np.int16)
        for w in range(NW):
            g = k * NW + w
            s, e = starts[g], starts[g + 1]
            ssrc = srcs[s:e]
            sdst = dsts[s:e]
            ka = int(lo_cnt[k, w])
            nlo, nhi, b_ = NLO[w], NHI[w], B[w]
            # slot layout: [lo edges | pad to nlo*128 | hi edges | pad]
            slots_src = np.zeros(b_ * 128, np.int32)
            slots_dstl = np.full(b_ * 128, 255.0, np.float32)
            slots_dstg = np.zeros(b_ * 128, np.int32)
            slots_src[:ka] = ssrc[:ka]
            slots_dstl[:ka] = (sdst[:ka] % WIN).astype(np.float32)
            slots_dstg[:ka] = sdst[:ka] - k * NPC
            hs = nlo * 128
            nh = len(ssrc) - ka
            slots_src[hs:hs + nh] = ssrc[ka:] - SPLIT
            slots_dstl[hs:hs + nh] = (sdst[ka:] % WIN).astype(np.float32)
            slots_dstg[hs:hs + nh] = sdst[ka:] - k * NPC
            if nlo:
                lo_tab[:, CLO[w]:CLO[w] + nlo * 8] = _wrap16(
                    slots_src[:nlo * 128], nlo * 8)
            if nhi:
                hi_tab[:, CHI[w]:CHI[w] + nhi * 8] = _wrap16(
                    slots_src[hs:hs + nhi * 128], nhi * 8)
            xr_tab[:, C[w] * 8:(C[w] + b_) * 8] = _wrap16(slots_dstg, b_ * 8)
            dstl[:, C[w]:C[w] + b_] = slots_dstl.reshape(b_, 128).T

        in_maps.append({
            "xt": xt,
            "w1t": w1t, "b1r": b1cat.astype(np.float16)[None, :],
            "w2t": w2t, "b2r": b2cat.astype(np.float16)[None, :],
            "att1B": att1B, "att2B": att2B,
            "bias1c": bias1c, "bias2b": bias2b,
            "iota": iota_row,
            "dstlT": dstl,
            "xlloT": lo_tab, "xlhiT": hi_tab, "xrT": xr_tab,
        })
    return in_maps, meta


def gat_forward(inputs, *, runner=None, dbg=False, want_results=None):
    in_maps, meta = prepare(inputs)
    nc = build_program(meta, dbg=dbg)
    if runner is not None:
        results = runner(nc, in_maps)
    else:
        results = run_bass_kernel_spmd(nc, in_maps, list(range(NCORES))).results
    if want_results is not None:
        want_results.extend(results)
    out = np.concatenate([results[k]["out_ext"] for k in range(NCORES)], axis=0)
    return np.ascontiguousarray(out[:N_NODES])


def kernel(**inputs):
    return gat_forward(inputs)
